# revision 7
# baseline (speedup 1.0000x reference)
"""MoE (MiMo-V2) kernel for 8x Trainium2 NeuronCores.

Strategy (expert-parallel, per the sharding hint):
  - Host: grouped-topk routing (exact replica of the reference gate, run in
    fp32 on jax-cpu), then tokens are gathered per expert into fixed-capacity
    segments. Each of the 8 cores owns 8 experts (snake assignment by
    descending token count -> slot capacity = max over cores, near-minimal
    SPMD padding; caps rounded to 32).
  - Device (Bass/Tile, one SPMD program): per expert, stream token blocks of
    <=1024 through gate/up matmuls (bf16, fp32 PSUM) with explicit LDWEIGHTS
    + weight-elided matmuls (the self-loading LDW+MM pair costs ~46ns extra
    per MM on HW; the explicit form streams at the 213ns/MM roofline),
    silu*mul on ACT/DVE into an [I, tokens] act tile, then the down matmul
    (act stationary, wd moving, also LDW-elided), scale rows by the combine
    weights, write bf16 rows out.
  - Host: scatter-add the gathered per-expert rows into the [T, H] output.
"""

import numpy as np
import ml_dtypes

T, H, E, I, K, G, KG = 16384, 1024, 64, 768, 8, 8, 4
P = 128
NCORES = 8
EPC = E // NCORES  # experts per core
HC = H // P  # 8 contraction chunks for gate/up
IC = I // P  # 6 contraction chunks for down
I2 = 2 * I  # fused gate+up output width
CAP_ROUND = 1
BLK = 1024  # token block (2 PSUM sub-blocks of 512 per g/u chunk)

BF16 = ml_dtypes.bfloat16

_program_cache = {}
_weights_cache = {}
last_results = None  # BassKernelResults of the most recent launch (for test.py)


def _routing_np(hidden, gate_w, bias):
    """Numpy fallback for the grouped-topk gate (same ops/tie rules)."""
    logits = hidden.astype(np.float32) @ gate_w.T.astype(np.float32)
    scores = 1.0 / (1.0 + np.exp(-logits))
    s_choice = scores + bias[None, :].astype(np.float32)
    t, e = scores.shape
    grouped = s_choice.reshape(t, G, e // G)
    top2 = np.sort(grouped, axis=-1)[..., -2:]
    group_scores = top2.sum(-1)
    gidx = np.argsort(-group_scores, axis=1, kind="stable")[:, :KG]
    gmask = np.zeros((t, G), np.float32)
    gmask[np.arange(t)[:, None], gidx] = 1.0
    emask = np.repeat(gmask, e // G, axis=1)
    masked = np.where(emask > 0, s_choice, -np.inf)
    topk_idx = np.argsort(-masked, axis=1, kind="stable")[:, :K].astype(np.int32)
    topk_w = np.take_along_axis(scores, topk_idx, axis=1)
    topk_w = topk_w / (topk_w.sum(-1, keepdims=True) + 1e-20)
    return topk_idx, topk_w.astype(np.float32)


def _routing(hidden, gate_w, bias):
    """Exact replica of reference._grouped_topk on jax-cpu (fp32)."""
    try:
        import jax
        import jax.numpy as jnp

        cpu = jax.devices("cpu")[0]
    except Exception:
        return _routing_np(np.asarray(hidden), np.asarray(gate_w), np.asarray(bias))
    with jax.default_device(cpu):
        hidden = jnp.asarray(np.asarray(hidden), jnp.float32)
        gate_w = jnp.asarray(np.asarray(gate_w), jnp.float32)
        bias = jnp.asarray(np.asarray(bias), jnp.float32)
        logits = hidden @ gate_w.T
        scores = jax.nn.sigmoid(logits)
        s_choice = scores + bias[None, :]
        t, e = scores.shape
        grouped = s_choice.reshape(t, G, e // G)
        top2, _ = jax.lax.top_k(grouped, 2)
        group_scores = top2.sum(-1)
        _, gidx = jax.lax.top_k(group_scores, KG)
        gmask = jnp.zeros((t, G), jnp.float32).at[jnp.arange(t)[:, None], gidx].set(1.0)
        emask = jnp.repeat(gmask, e // G, axis=1)
        masked = jnp.where(emask > 0, s_choice, -jnp.inf)
        _, topk_idx = jax.lax.top_k(masked, K)
        topk_w = jnp.take_along_axis(scores, topk_idx, axis=1)
        topk_w = topk_w / (topk_w.sum(-1, keepdims=True) + 1e-20)
        return np.asarray(topk_idx), np.asarray(topk_w, np.float32)


def _blocks_of(cap, ascending, head_split=False):
    """Decompose a slot capacity into token blocks of <=BLK."""
    rem = cap % BLK
    bl = ([rem] if rem else []) + [BLK] * (cap // BLK)
    if head_split and bl[0] > 256:
        # tiny first block so the first matmul starts ~10us earlier
        bl = [128, bl[0] - 128] + bl[1:]
    if not ascending:
        bl = bl[::-1]
    return bl


def _build_program(slot_blocks):
    """One SPMD Bass program. slot_blocks[j] is the token-block decomposition
    of local-expert slot j; slots have (generally different) fixed capacities
    shared by all cores."""
    import concourse.mybir as mybir
    from concourse import bacc
    from concourse.tile import TileContext

    caps = [sum(b) for b in slot_blocks]
    seg_off = np.zeros(EPC + 1, np.int64)
    np.cumsum(caps, out=seg_off[1:])
    NC = int(seg_off[-1])
    bf = mybir.dt.bfloat16
    f32 = mybir.dt.float32
    Silu = mybir.ActivationFunctionType.Silu
    mult = mybir.AluOpType.mult

    nc = bacc.Bacc("TRN2", target_bir_lowering=False, debug=False, num_devices=NCORES)
    xgt = nc.dram_tensor("xgt", [H, NC], bf, kind="ExternalInput").ap()
    wgu = nc.dram_tensor("wgu", [EPC, H, I2], bf, kind="ExternalInput").ap()
    wd = nc.dram_tensor("wd", [EPC, I, H], bf, kind="ExternalInput").ap()
    cv = nc.dram_tensor("cv", [NC, 1], f32, kind="ExternalInput").ap()
    head_bn = slot_blocks[0][0] if slot_blocks[0][0] <= P else 0
    xh = (
        nc.dram_tensor("xh", [P, HC * P], bf, kind="ExternalInput").ap()
        if head_bn
        else None
    )
    g = nc.dram_tensor("g", [NC, H], bf, kind="ExternalOutput").ap()

    def mm_group(tensor_eng, w_ap, mms):
        """Explicit LDWEIGHTS + weight-elided matmuls sharing it."""
        tensor_eng.ldweights(w_ap)
        for out_ap, rhs_ap, start, stop in mms:
            mm = tensor_eng.matmul(
                out=out_ap, lhsT=w_ap, rhs=rhs_ap, start=start, stop=stop
            )
            mm.ins.ldweights = False

    with TileContext(nc) as tc:
        with (
            tc.tile_pool(name="wpool", bufs=2) as wpool,
            tc.tile_pool(name="xpool", bufs=2) as xpool,
            tc.tile_pool(name="apool", bufs=2) as apool,
            tc.tile_pool(name="spool", bufs=2) as spool,
            tc.tile_pool(name="opool", bufs=4) as opool,
            tc.tile_pool(name="cpool", bufs=4) as cpool,
            tc.tile_pool(name="psgu", bufs=1, space="PSUM") as psgu,
            tc.tile_pool(name="pso", bufs=2, space="PSUM") as pso,
        ):
            xgt_r = xgt.rearrange("(c p) t -> p c t", p=P)  # [128, HC, NC]
            for ei in range(EPC):
                wgu_r = wgu[ei].rearrange("(c p) i -> c p i", p=P)
                wd_r = wd[ei].rearrange("(c p) h -> c p h", p=P)
                blocks = slot_blocks[ei]

                def load_wgu():
                    tiles = []
                    for hc in range(HC):
                        w = wpool.tile([P, I2], bf, tag=f"wgu{hc}")
                        nc.sync.dma_start(out=w[:], in_=wgu_r[hc])
                        tiles.append(w)
                    return tiles

                def load_wd():
                    tiles = []
                    for ic in range(IC):
                        w = wpool.tile([P, H], bf, tag=f"wd{ic}")
                        nc.sync.dma_start(out=w[:], in_=wd_r[ic])
                        tiles.append(w)
                    return tiles

                def load_x(s, bn):
                    tiles = []
                    for hc in range(HC):
                        xt = xpool.tile([P, BLK], bf, tag=f"xg{hc}")
                        nc.sync.dma_start(out=xt[:, :bn], in_=xgt_r[:, hc, s : s + bn])
                        tiles.append(xt)
                    return tiles

                if ei == 0:
                    # head-latency: the first matmul needs only wgu tile 0 and
                    # the (tiny) first block's x; emit those DMAs first, the x
                    # as ONE transfer from the hc-major head copy
                    w0t = wpool.tile([P, I2], bf, tag="wgu0", name="wgu0")
                    nc.sync.dma_start(out=w0t[:], in_=wgu_r[0])
                    s0 = int(seg_off[0])
                    bn0 = blocks[0]
                    if head_bn:
                        xh_t = xpool.tile([P, HC * P], bf, tag="xh", name="xh")
                        nc.sync.dma_start(out=xh_t[:], in_=xh)
                        xg_sb = None
                    else:
                        xg_sb = load_x(s0, bn0)
                    wgu_sb = [w0t]
                    for hc in range(1, HC):
                        w = wpool.tile([P, I2], bf, tag=f"wgu{hc}", name=f"wgu{hc}")
                        nc.sync.dma_start(out=w[:], in_=wgu_r[hc])
                        wgu_sb.append(w)
                    wd_sb = load_wd()
                else:
                    wgu_sb = load_wgu()
                    wd_sb = load_wd()

                off = 0
                for bi, bn in enumerate(blocks):
                    s = int(seg_off[ei]) + off
                    if not (ei == 0 and bi == 0):
                        xg_sb = load_x(s, bn)
                    # sub-blocks of <=512 within this block
                    sbs = [
                        (q * 512, min(512, bn - q * 512))
                        for q in range((bn + 511) // 512)
                    ]
                    act_sb = apool.tile([P, IC, BLK], bf, tag="act")
                    # --- gate/up: chunk pairs (gate j, up j); one 2-bank
                    # psum tile per phase (each MM writes within one bank,
                    # ACT/DVE read across banks -> one consumer, one seam wait)
                    for jj in range(IC):
                        pg = psgu.tile([P, BLK], f32, tag="pg", name="pg")
                        pu = psgu.tile([P, BLK], f32, tag="pu", name="pu")
                        for which, ps_tile in (("g", pg), ("u", pu)):
                            base = jj * P if which == "g" else I + jj * P
                            for hc in range(HC):
                                mm_group(
                                    nc.tensor,
                                    wgu_sb[hc][:, base : base + P],
                                    [
                                        (
                                            ps_tile[:, q0 : q0 + qn],
                                            (
                                                xh_t[:, hc * P : hc * P + qn]
                                                if xg_sb is None
                                                else xg_sb[hc][:, q0 : q0 + qn]
                                            ),
                                            hc == 0,
                                            hc == HC - 1,
                                        )
                                        for (q0, qn) in sbs
                                    ],
                                )
                        sg = spool.tile([P, BLK], f32, tag="sg", name="sg")
                        nc.scalar.activation(
                            out=sg[:, :bn], in_=pg[:, :bn], func=Silu
                        )
                        nc.vector.tensor_tensor(
                            out=act_sb[:, jj, :bn],
                            in0=sg[:, :bn],
                            in1=pu[:, :bn],
                            op=mult,
                        )
                    # --- down: token tiles of <=128 ---
                    nt = (bn + P - 1) // P
                    for ts in range(nt):
                        t0 = ts * P
                        tn = min(P, bn - t0)
                        ct = cpool.tile([P, 1], f32, tag="ct")
                        nc.sync.dma_start(
                            out=ct[:tn], in_=cv[s + t0 : s + t0 + tn, :]
                        )
                        po = pso.tile([P, H], f32, tag="po", name="po")
                        for i in range(IC):
                            mm_group(
                                nc.tensor,
                                act_sb[:, i, t0 : t0 + tn],
                                [
                                    (
                                        po[:tn, nh * 512 : (nh + 1) * 512],
                                        wd_sb[i][:, nh * 512 : (nh + 1) * 512],
                                        i == 0,
                                        i == IC - 1,
                                    )
                                    for nh in range(2)
                                ],
                            )
                        ob = opool.tile([P, H], bf, tag="ob")
                        nc.vector.tensor_tensor(
                            out=ob[:tn, :],
                            in0=po[:tn, :],
                            in1=ct[:tn].to_broadcast([tn, H]),
                            op=mult,
                        )
                        nc.sync.dma_start(
                            out=g[s + t0 : s + t0 + tn, :], in_=ob[:tn, :]
                        )
                    off += bn
    nc.compile()
    return nc


def kernel(hidden_states, gate_weight, correction_bias, w_gate, w_up, w_down):
    global last_results
    from concourse.bass_utils import run_bass_kernel_spmd

    hidden = np.ascontiguousarray(np.asarray(hidden_states, np.float32))
    w_gate = np.asarray(w_gate, np.float32)
    w_up = np.asarray(w_up, np.float32)
    w_down = np.asarray(w_down, np.float32)

    topk_idx, topk_w = _routing(hidden, gate_weight, correction_bias)

    # Per-expert token lists (ascending), via stable sort of the (token, k) pairs.
    flat_e = topk_idx.ravel()
    order = np.argsort(flat_e, kind="stable")
    tokens_sorted = (order // K).astype(np.int64)
    weights_sorted = topk_w.ravel()[order]
    counts = np.bincount(flat_e, minlength=E)
    starts = np.zeros(E + 1, np.int64)
    np.cumsum(counts, out=starts[1:])

    # Snake-assign experts to cores by descending token count, slot j of core c
    # = j-th largest expert of that core. Slot capacity = max over cores of
    # that order statistic = the (8j)-th global order statistic; this grouping
    # minimizes sum-of-slot-maxes over all SPMD-valid assignments.
    rank = np.argsort(-counts, kind="stable")
    core_experts = [[] for _ in range(NCORES)]
    for r, e in enumerate(rank):
        blk, pos = divmod(r, NCORES)
        c = pos if blk % 2 == 0 else NCORES - 1 - pos
        core_experts[c].append(int(e))
    slot_expert = np.array(core_experts)  # [NCORES, EPC], desc count order
    sorted_counts = counts[slot_expert]
    caps = ((sorted_counts.max(axis=0) + CAP_ROUND - 1) // CAP_ROUND) * CAP_ROUND
    caps = np.maximum(caps, CAP_ROUND)
    slot_blocks = []
    for j in range(EPC):
        # slot 0: ascending (small first block -> early first matmul);
        # last slot: descending (small last block -> short tail)
        slot_blocks.append(tuple(_blocks_of(int(caps[j]), ascending=(j < EPC - 1), head_split=(j == 0))))

    print(f"[kernel] expert counts min/mean/max: {counts.min()}/{counts.mean():.0f}/{counts.max()}; "
          f"slot caps {list(map(int, caps))} sum {int(caps.sum())}")
    key = tuple(slot_blocks)
    if key not in _program_cache:
        _program_cache[key] = _build_program([list(b) for b in slot_blocks])
    nc = _program_cache[key]

    seg_off = np.zeros(EPC + 1, np.int64)
    np.cumsum(caps, out=seg_off[1:])
    NC = int(seg_off[-1])

    wkey = (
        slot_expert.tobytes(),
        float(w_gate[0, 0, 0]),
        float(w_up[0, 0, 0]),
        float(w_down[-1, -1, -1]),
    )
    cached_w = _weights_cache.get(wkey)
    if cached_w is None:
        cached_w = []
        for c in range(NCORES):
            wgu_c = np.empty((EPC, H, I2), BF16)
            wd_c = np.empty((EPC, I, H), BF16)
            for j in range(EPC):
                e = int(slot_expert[c, j])
                wgu_c[j, :, :I] = w_gate[e].T.astype(BF16)
                wgu_c[j, :, I:] = w_up[e].T.astype(BF16)
                wd_c[j] = w_down[e].T.astype(BF16)
            cached_w.append((wgu_c, wd_c))
        _weights_cache.clear()
        _weights_cache[wkey] = cached_w

    hidden_bf_t = np.ascontiguousarray(hidden.T).astype(BF16)  # [H, T]
    in_maps = []
    tok_lists = []
    for c in range(NCORES):
        perm = np.zeros(NC, np.int64)
        cw = np.zeros((NC, 1), np.float32)
        toks_c = []
        for j in range(EPC):
            e = int(slot_expert[c, j])
            n = counts[e]
            s = int(seg_off[j])
            te = tokens_sorted[starts[e] : starts[e] + n]
            perm[s : s + n] = te
            cw[s : s + n, 0] = weights_sorted[starts[e] : starts[e] + n]
            toks_c.append(te)
        tok_lists.append(toks_c)
        xgt = hidden_bf_t[:, perm]
        wgu_c, wd_c = cached_w[c]
        im = {"xgt": xgt, "wgu": wgu_c, "wd": wd_c, "cv": cw}
        bn0 = slot_blocks[0][0]
        if bn0 <= P:
            xh_c = np.zeros((P, HC * P), BF16)
            for hc in range(HC):
                xh_c[:, hc * P : hc * P + bn0] = xgt[hc * P : (hc + 1) * P, :bn0]
            im["xh"] = xh_c
        in_maps.append(im)

    last_results = run_bass_kernel_spmd(nc, in_maps, list(range(NCORES)))

    out = np.zeros((T, H), np.float32)
    for c in range(NCORES):
        gc = last_results.results[c]["g"]
        for j in range(EPC):
            e = int(slot_expert[c, j])
            n = counts[e]
            s = int(seg_off[j])
            out[tok_lists[c][j]] += gc[s : s + n].astype(np.float32)
    return out


# revision 8
# speedup vs baseline: 1.0233x; 1.0233x over previous
"""MoE (MiMo-V2) kernel for 8x Trainium2 NeuronCores.

Strategy (expert-parallel, per the sharding hint):
  - Host: grouped-topk routing (exact replica of the reference gate, run in
    fp32 on jax-cpu), then tokens are gathered per expert into fixed-capacity
    segments. Each of the 8 cores owns 8 experts (snake assignment by
    descending token count -> slot capacity = max over cores, near-minimal
    SPMD padding; caps rounded to 32).
  - Device (Bass/Tile, one SPMD program): per expert, stream token blocks of
    <=1024 through gate/up matmuls (bf16, fp32 PSUM) with explicit LDWEIGHTS
    + weight-elided matmuls (the self-loading LDW+MM pair costs ~46ns extra
    per MM on HW; the explicit form streams at the 213ns/MM roofline),
    silu*mul on ACT/DVE into an [I, tokens] act tile, then the down matmul
    (act stationary, wd moving, also LDW-elided), scale rows by the combine
    weights, write bf16 rows out.
  - Host: scatter-add the gathered per-expert rows into the [T, H] output.
"""

import numpy as np
import ml_dtypes

T, H, E, I, K, G, KG = 16384, 1024, 64, 768, 8, 8, 4
P = 128
NCORES = 8
EPC = E // NCORES  # experts per core
HC = H // P  # 8 contraction chunks for gate/up
IC = I // P  # 6 contraction chunks for down
I2 = 2 * I  # fused gate+up output width
CAP_ROUND = 1
BLK = 1024  # token block (2 PSUM sub-blocks of 512 per g/u chunk)

BF16 = ml_dtypes.bfloat16

_program_cache = {}
_weights_cache = {}
last_results = None  # BassKernelResults of the most recent launch (for test.py)


def _routing_np(hidden, gate_w, bias):
    """Numpy fallback for the grouped-topk gate (same ops/tie rules)."""
    logits = hidden.astype(np.float32) @ gate_w.T.astype(np.float32)
    scores = 1.0 / (1.0 + np.exp(-logits))
    s_choice = scores + bias[None, :].astype(np.float32)
    t, e = scores.shape
    grouped = s_choice.reshape(t, G, e // G)
    top2 = np.sort(grouped, axis=-1)[..., -2:]
    group_scores = top2.sum(-1)
    gidx = np.argsort(-group_scores, axis=1, kind="stable")[:, :KG]
    gmask = np.zeros((t, G), np.float32)
    gmask[np.arange(t)[:, None], gidx] = 1.0
    emask = np.repeat(gmask, e // G, axis=1)
    masked = np.where(emask > 0, s_choice, -np.inf)
    topk_idx = np.argsort(-masked, axis=1, kind="stable")[:, :K].astype(np.int32)
    topk_w = np.take_along_axis(scores, topk_idx, axis=1)
    topk_w = topk_w / (topk_w.sum(-1, keepdims=True) + 1e-20)
    return topk_idx, topk_w.astype(np.float32)


def _routing(hidden, gate_w, bias):
    """Exact replica of reference._grouped_topk on jax-cpu (fp32)."""
    try:
        import jax
        import jax.numpy as jnp

        cpu = jax.devices("cpu")[0]
    except Exception:
        return _routing_np(np.asarray(hidden), np.asarray(gate_w), np.asarray(bias))
    with jax.default_device(cpu):
        hidden = jnp.asarray(np.asarray(hidden), jnp.float32)
        gate_w = jnp.asarray(np.asarray(gate_w), jnp.float32)
        bias = jnp.asarray(np.asarray(bias), jnp.float32)
        logits = hidden @ gate_w.T
        scores = jax.nn.sigmoid(logits)
        s_choice = scores + bias[None, :]
        t, e = scores.shape
        grouped = s_choice.reshape(t, G, e // G)
        top2, _ = jax.lax.top_k(grouped, 2)
        group_scores = top2.sum(-1)
        _, gidx = jax.lax.top_k(group_scores, KG)
        gmask = jnp.zeros((t, G), jnp.float32).at[jnp.arange(t)[:, None], gidx].set(1.0)
        emask = jnp.repeat(gmask, e // G, axis=1)
        masked = jnp.where(emask > 0, s_choice, -jnp.inf)
        _, topk_idx = jax.lax.top_k(masked, K)
        topk_w = jnp.take_along_axis(scores, topk_idx, axis=1)
        topk_w = topk_w / (topk_w.sum(-1, keepdims=True) + 1e-20)
        return np.asarray(topk_idx), np.asarray(topk_w, np.float32)


def _blocks_of(cap, ascending, head_split=False):
    """Decompose a slot capacity into token blocks of <=BLK."""
    rem = cap % BLK
    bl = ([rem] if rem else []) + [BLK] * (cap // BLK)
    if head_split and bl[0] > 256:
        # tiny first block so the first matmul starts ~10us earlier
        bl = [128, bl[0] - 128] + bl[1:]
    if not ascending:
        bl = bl[::-1]
    return bl


def _build_program(slot_blocks):
    """One SPMD Bass program. slot_blocks[j] is the token-block decomposition
    of local-expert slot j; slots have (generally different) fixed capacities
    shared by all cores."""
    import concourse.mybir as mybir
    from concourse import bacc
    from concourse.tile import TileContext

    caps = [sum(b) for b in slot_blocks]
    seg_off = np.zeros(EPC + 1, np.int64)
    np.cumsum(caps, out=seg_off[1:])
    NC = int(seg_off[-1])
    bf = mybir.dt.bfloat16
    f32 = mybir.dt.float32
    Silu = mybir.ActivationFunctionType.Silu
    mult = mybir.AluOpType.mult

    nc = bacc.Bacc("TRN2", target_bir_lowering=False, debug=False, num_devices=NCORES)
    xgt = nc.dram_tensor("xgt", [H, NC], bf, kind="ExternalInput").ap()
    wgu = nc.dram_tensor("wgu", [EPC, H, I2], bf, kind="ExternalInput").ap()
    wd = nc.dram_tensor("wd", [EPC, I, H], bf, kind="ExternalInput").ap()
    cv = nc.dram_tensor("cv", [NC, 1], f32, kind="ExternalInput").ap()
    head_bn = slot_blocks[0][0] if slot_blocks[0][0] <= P else 0
    xh = (
        nc.dram_tensor("xh", [P, HC * P], bf, kind="ExternalInput").ap()
        if head_bn
        else None
    )
    g = nc.dram_tensor("g", [NC, H], bf, kind="ExternalOutput").ap()

    def mm_group(tensor_eng, w_ap, mms):
        """Explicit LDWEIGHTS + weight-elided matmuls sharing it."""
        tensor_eng.ldweights(w_ap)
        for out_ap, rhs_ap, start, stop in mms:
            mm = tensor_eng.matmul(
                out=out_ap, lhsT=w_ap, rhs=rhs_ap, start=start, stop=stop
            )
            mm.ins.ldweights = False

    with TileContext(nc) as tc:
        with (
            tc.tile_pool(name="wpool", bufs=2) as wpool,
            tc.tile_pool(name="xpool", bufs=2) as xpool,
            tc.tile_pool(name="apool", bufs=2) as apool,
            tc.tile_pool(name="spool", bufs=2) as spool,
            tc.tile_pool(name="opool", bufs=4) as opool,
            tc.tile_pool(name="cpool", bufs=4) as cpool,
            tc.tile_pool(name="psgu", bufs=1, space="PSUM") as psgu,
            tc.tile_pool(name="pso", bufs=2, space="PSUM") as pso,
        ):
            xgt_r = xgt.rearrange("(c p) t -> p c t", p=P)  # [128, HC, NC]
            for ei in range(EPC):
                wgu_r = wgu[ei].rearrange("(c p) i -> c p i", p=P)
                wd_r = wd[ei].rearrange("(c p) h -> c p h", p=P)
                blocks = slot_blocks[ei]

                def load_wgu():
                    tiles = []
                    for hc in range(HC):
                        w = wpool.tile([P, I2], bf, tag=f"wgu{hc}")
                        nc.sync.dma_start(out=w[:], in_=wgu_r[hc])
                        tiles.append(w)
                    return tiles

                def load_wd():
                    tiles = []
                    for ic in range(IC):
                        w = wpool.tile([P, H], bf, tag=f"wd{ic}")
                        nc.sync.dma_start(out=w[:], in_=wd_r[ic])
                        tiles.append(w)
                    return tiles

                def load_x(s, bn):
                    tiles = []
                    for hc in range(HC):
                        xt = xpool.tile([P, BLK], bf, tag=f"xg{hc}")
                        nc.sync.dma_start(out=xt[:, :bn], in_=xgt_r[:, hc, s : s + bn])
                        tiles.append(xt)
                    return tiles

                if ei == 0:
                    # head-latency: the first matmul needs only wgu tile 0 and
                    # the (tiny) first block's x; emit those DMAs first, the x
                    # as ONE transfer from the hc-major head copy
                    w0t = wpool.tile([P, I2], bf, tag="wgu0", name="wgu0")
                    nc.sync.dma_start(out=w0t[:], in_=wgu_r[0])
                    s0 = int(seg_off[0])
                    bn0 = blocks[0]
                    if head_bn:
                        xh_t = xpool.tile([P, HC * P], bf, tag="xh", name="xh")
                        nc.sync.dma_start(out=xh_t[:], in_=xh)
                        xg_sb = None
                    else:
                        xg_sb = load_x(s0, bn0)
                    wgu_sb = [w0t]
                    for hc in range(1, HC):
                        w = wpool.tile([P, I2], bf, tag=f"wgu{hc}", name=f"wgu{hc}")
                        nc.sync.dma_start(out=w[:], in_=wgu_r[hc])
                        wgu_sb.append(w)
                    wd_sb = load_wd()
                else:
                    wgu_sb = load_wgu()
                    wd_sb = load_wd()

                off = 0
                for bi, bn in enumerate(blocks):
                    s = int(seg_off[ei]) + off
                    if not (ei == 0 and bi == 0):
                        xg_sb = load_x(s, bn)
                    # sub-blocks of <=512 within this block
                    sbs = [
                        (q * 512, min(512, bn - q * 512))
                        for q in range((bn + 511) // 512)
                    ]
                    act_sb = apool.tile([P, IC, BLK], bf, tag="act")
                    # --- gate/up: chunk pairs (gate j, up j) ---
                    for jj in range(IC):
                        pg = [
                            psgu.tile([P, 512], f32, tag=f"pg{si}", name=f"pg{si}")
                            for si in range(len(sbs))
                        ]
                        pu = [
                            psgu.tile([P, 512], f32, tag=f"pu{si}", name=f"pu{si}")
                            for si in range(len(sbs))
                        ]
                        for which, ps_tiles in (("g", pg), ("u", pu)):
                            base = jj * P if which == "g" else I + jj * P
                            for hc in range(HC):
                                mm_group(
                                    nc.tensor,
                                    wgu_sb[hc][:, base : base + P],
                                    [
                                        (
                                            ps_tiles[si][:, :qn],
                                            (
                                                xh_t[:, hc * P : hc * P + qn]
                                                if xg_sb is None
                                                else xg_sb[hc][:, q0 : q0 + qn]
                                            ),
                                            hc == 0,
                                            hc == HC - 1,
                                        )
                                        for si, (q0, qn) in enumerate(sbs)
                                    ],
                                )
                        for si, (q0, qn) in enumerate(sbs):
                            sg = spool.tile([P, 512], f32, tag=f"sg{si}", name=f"sg{si}")
                            nc.scalar.activation(
                                out=sg[:, :qn], in_=pg[si][:, :qn], func=Silu
                            )
                            nc.vector.tensor_tensor(
                                out=act_sb[:, jj, q0 : q0 + qn],
                                in0=sg[:, :qn],
                                in1=pu[si][:, :qn],
                                op=mult,
                            )
                    # --- down: token tiles of <=128 ---
                    nt = (bn + P - 1) // P
                    for ts in range(nt):
                        t0 = ts * P
                        tn = min(P, bn - t0)
                        ct = cpool.tile([P, 1], f32, tag="ct")
                        nc.sync.dma_start(
                            out=ct[:tn], in_=cv[s + t0 : s + t0 + tn, :]
                        )
                        po = pso.tile([P, H], f32, tag="po", name="po")
                        for i in range(IC):
                            mm_group(
                                nc.tensor,
                                act_sb[:, i, t0 : t0 + tn],
                                [
                                    (
                                        po[:tn, nh * 512 : (nh + 1) * 512],
                                        wd_sb[i][:, nh * 512 : (nh + 1) * 512],
                                        i == 0,
                                        i == IC - 1,
                                    )
                                    for nh in range(2)
                                ],
                            )
                        ob = opool.tile([P, H], bf, tag="ob")
                        nc.vector.tensor_tensor(
                            out=ob[:tn, :],
                            in0=po[:tn, :],
                            in1=ct[:tn].to_broadcast([tn, H]),
                            op=mult,
                        )
                        nc.sync.dma_start(
                            out=g[s + t0 : s + t0 + tn, :], in_=ob[:tn, :]
                        )
                    off += bn
    nc.compile()
    return nc


def kernel(hidden_states, gate_weight, correction_bias, w_gate, w_up, w_down):
    global last_results
    from concourse.bass_utils import run_bass_kernel_spmd

    hidden = np.ascontiguousarray(np.asarray(hidden_states, np.float32))
    w_gate = np.asarray(w_gate, np.float32)
    w_up = np.asarray(w_up, np.float32)
    w_down = np.asarray(w_down, np.float32)

    topk_idx, topk_w = _routing(hidden, gate_weight, correction_bias)

    # Per-expert token lists (ascending), via stable sort of the (token, k) pairs.
    flat_e = topk_idx.ravel()
    order = np.argsort(flat_e, kind="stable")
    tokens_sorted = (order // K).astype(np.int64)
    weights_sorted = topk_w.ravel()[order]
    counts = np.bincount(flat_e, minlength=E)
    starts = np.zeros(E + 1, np.int64)
    np.cumsum(counts, out=starts[1:])

    # Snake-assign experts to cores by descending token count, slot j of core c
    # = j-th largest expert of that core. Slot capacity = max over cores of
    # that order statistic = the (8j)-th global order statistic; this grouping
    # minimizes sum-of-slot-maxes over all SPMD-valid assignments.
    rank = np.argsort(-counts, kind="stable")
    core_experts = [[] for _ in range(NCORES)]
    for r, e in enumerate(rank):
        blk, pos = divmod(r, NCORES)
        c = pos if blk % 2 == 0 else NCORES - 1 - pos
        core_experts[c].append(int(e))
    slot_expert = np.array(core_experts)  # [NCORES, EPC], desc count order
    sorted_counts = counts[slot_expert]
    caps = ((sorted_counts.max(axis=0) + CAP_ROUND - 1) // CAP_ROUND) * CAP_ROUND
    caps = np.maximum(caps, CAP_ROUND)
    slot_blocks = []
    for j in range(EPC):
        # slot 0: ascending (small first block -> early first matmul);
        # last slot: descending (small last block -> short tail)
        slot_blocks.append(tuple(_blocks_of(int(caps[j]), ascending=(j < EPC - 1), head_split=(j == 0))))

    print(f"[kernel] expert counts min/mean/max: {counts.min()}/{counts.mean():.0f}/{counts.max()}; "
          f"slot caps {list(map(int, caps))} sum {int(caps.sum())}")
    key = tuple(slot_blocks)
    if key not in _program_cache:
        _program_cache[key] = _build_program([list(b) for b in slot_blocks])
    nc = _program_cache[key]

    seg_off = np.zeros(EPC + 1, np.int64)
    np.cumsum(caps, out=seg_off[1:])
    NC = int(seg_off[-1])

    wkey = (
        slot_expert.tobytes(),
        float(w_gate[0, 0, 0]),
        float(w_up[0, 0, 0]),
        float(w_down[-1, -1, -1]),
    )
    cached_w = _weights_cache.get(wkey)
    if cached_w is None:
        cached_w = []
        for c in range(NCORES):
            wgu_c = np.empty((EPC, H, I2), BF16)
            wd_c = np.empty((EPC, I, H), BF16)
            for j in range(EPC):
                e = int(slot_expert[c, j])
                wgu_c[j, :, :I] = w_gate[e].T.astype(BF16)
                wgu_c[j, :, I:] = w_up[e].T.astype(BF16)
                wd_c[j] = w_down[e].T.astype(BF16)
            cached_w.append((wgu_c, wd_c))
        _weights_cache.clear()
        _weights_cache[wkey] = cached_w

    hidden_bf_t = np.ascontiguousarray(hidden.T).astype(BF16)  # [H, T]
    in_maps = []
    tok_lists = []
    for c in range(NCORES):
        perm = np.zeros(NC, np.int64)
        cw = np.zeros((NC, 1), np.float32)
        toks_c = []
        for j in range(EPC):
            e = int(slot_expert[c, j])
            n = counts[e]
            s = int(seg_off[j])
            te = tokens_sorted[starts[e] : starts[e] + n]
            perm[s : s + n] = te
            cw[s : s + n, 0] = weights_sorted[starts[e] : starts[e] + n]
            toks_c.append(te)
        tok_lists.append(toks_c)
        xgt = hidden_bf_t[:, perm]
        wgu_c, wd_c = cached_w[c]
        im = {"xgt": xgt, "wgu": wgu_c, "wd": wd_c, "cv": cw}
        bn0 = slot_blocks[0][0]
        if bn0 <= P:
            xh_c = np.zeros((P, HC * P), BF16)
            for hc in range(HC):
                xh_c[:, hc * P : hc * P + bn0] = xgt[hc * P : (hc + 1) * P, :bn0]
            im["xh"] = xh_c
        in_maps.append(im)

    last_results = run_bass_kernel_spmd(nc, in_maps, list(range(NCORES)))

    out = np.zeros((T, H), np.float32)
    for c in range(NCORES):
        gc = last_results.results[c]["g"]
        for j in range(EPC):
            e = int(slot_expert[c, j])
            n = counts[e]
            s = int(seg_off[j])
            out[tok_lists[c][j]] += gc[s : s + n].astype(np.float32)
    return out
